# revision 29
# baseline (speedup 1.0000x reference)
"""Trainium2 Bass kernel for nn_FourScanBranch (4-direction Mamba scan over video).

Strategy:
- 8 cores; core c handles (batch = c // 2, L-half = c % 2). All sequence chunks
  are made independent via a short burn-in prefix (the scan decay is
  exp(-(s+1)*dt) with dt ~= 0.127, so influence dies to <1e-12 within ~128
  steps) -- no cross-core or cross-chunk serial dependency.
- Per core, the half-sequence (Lh = 49152) is processed as 8 interleaved
  chunks packed along SBUF partitions for the small-channel stages
  (4/8/16-channel convs + projections run 8 chunks wide), while the Mamba
  recurrence runs per chunk on the full 128 (d_inner x d_state) partition
  width via the DVE tensor_tensor_scan instruction.
- Host does the static center-out permutation gather/scatter (pure index
  shuffles baked from shapes) and the final 4-scan mean.
"""
import numpy as np
from contextlib import ExitStack

# ---------------- problem constants (hardcoded per contract) ----------------
B, NF, HH, WW = 4, 6, 128, 128
L = NF * HH * WW          # 98304
D_MODEL, D_INNER, D_STATE, D_CONV, DT_RANK = 4, 8, 16, 4, 1

N_CORES = 8
LH = L // 2               # 49152 per core
NCH = 8                   # chunks per core (packed along partitions)
LC = LH // NCH            # 6144
WBURN = 128               # burn-in columns per chunk
T = 512                   # matmul/PSUM sub-tile width
TSTEP = 1024              # step width (DVE/ACT/scan ops run this wide)
RL = WBURN + 4 + LH + 1   # per-channel row length of the gathered input buffer

# steps: (g0, n) in chunk-local time; step 0 is burn-in only
STEPS = [(-WBURN, WBURN)] + [(k * TSTEP, TSTEP) for k in range(LC // TSTEP)]


def _build_perms():
    yy, xx = np.meshgrid(np.arange(HH), np.arange(WW), indexing='ij')
    cy, cx = HH // 2, WW // 2
    d2 = (yy - cy) ** 2 + (xx - cx) ** 2
    sp = np.argsort(d2.reshape(-1), kind='stable')
    HW = HH * WW
    scanA = (np.arange(NF)[:, None] * HW + sp[None, :]).reshape(-1)
    fwd = np.arange(NF)
    bwd = fwd[::-1]
    tids = np.where((np.arange(HW) % 2 == 0)[:, None], fwd[None, :], bwd[None, :])
    scanB = (tids * HW + sp[:, None]).reshape(-1)
    return np.stack([scanA, scanB, scanA[::-1], scanB[::-1]], axis=0)  # (4, L)


def _build_consts(inputs):
    """Host-side constant matrices, keyed by dram tensor name."""
    f32 = np.float32
    dwconv_w = np.asarray(inputs['dwconv_w'], f32)   # (4,1,3)
    dwconv_b = np.asarray(inputs['dwconv_b'], f32)   # (4,)
    in_proj_w = np.asarray(inputs['in_proj_w'], f32)  # (16,4)
    conv1d_w = np.asarray(inputs['conv1d_w'], f32)   # (8,1,4)
    conv1d_b = np.asarray(inputs['conv1d_b'], f32)   # (8,)
    x_proj_w = np.asarray(inputs['x_proj_w'], f32)   # (33,8)
    dt_proj_w = np.asarray(inputs['dt_proj_w'], f32)  # (8,1)
    dt_proj_b = np.asarray(inputs['dt_proj_b'], f32)  # (8,)
    A_log = np.asarray(inputs['A_log'], f32)         # (8,16)
    Dp = np.asarray(inputs['Dp'], f32)               # (8,)
    out_proj_w = np.asarray(inputs['out_proj_w'], f32)  # (4,8)
    ln_g = np.asarray(inputs['ln_g'], f32)           # (4,)

    C = {}
    # dwconv diag matmuls (32 = 8 chunks x 4 ch, p = 4j+c)
    for k in range(3):
        m = np.zeros((32, 32), f32)
        np.fill_diagonal(m, np.tile(dwconv_w[:, 0, k], NCH))
        C[f'dw{k}'] = m
    C['dwb'] = np.tile(dwconv_b, NCH).reshape(32, 1)
    # in_proj: (32, 128): [4j+c, 8j+d]=W[d,c]; [4j+c, 64+8j+d]=W[8+d,c]
    m = np.zeros((32, 128), f32)
    for j in range(NCH):
        for c in range(4):
            for d in range(8):
                m[4 * j + c, 8 * j + d] = in_proj_w[d, c]
                m[4 * j + c, 64 + 8 * j + d] = in_proj_w[8 + d, c]
    C['Win'] = m
    # conv1d stacked matmuls: cvA = [diag(w0); diag(w2)], cvB = [diag(w1); diag(w3)]
    for nm, (ka, kb) in (('cvA', (0, 2)), ('cvB', (1, 3))):
        m = np.zeros((128, 64), f32)
        np.fill_diagonal(m[0:64], np.tile(conv1d_w[:, 0, ka], NCH))
        np.fill_diagonal(m[64:128], np.tile(conv1d_w[:, 0, kb], NCH))
        C[nm] = m
    C['cb'] = np.tile(conv1d_b, NCH).reshape(64, 1)
    # dt_pre fused: [8j+dd, 8j+d] = x_proj_w[0,dd] * dt_proj_w[d,0]
    m = np.zeros((64, 64), f32)
    blk = np.outer(x_proj_w[0, :], dt_proj_w[:, 0]).astype(f32)  # (dd, d)
    for j in range(NCH):
        m[8 * j:8 * j + 8, 8 * j:8 * j + 8] = blk
    C['DD'] = m
    C['dtb'] = np.tile(dt_proj_b, NCH).reshape(64, 1)
    # packed x_proj fused broadcasts: Bm128[16j+s, t] = sum_dd xp[1+s,dd]*xm2[8j+dd,t]
    m = np.zeros((64, 128), f32)
    mc = np.zeros((64, 128), f32)
    for j in range(NCH):
        for dd in range(8):
            for sspan in range(16):
                m[8 * j + dd, 16 * j + sspan] = x_proj_w[1 + sspan, dd]
                mc[8 * j + dd, 16 * j + sspan] = x_proj_w[17 + sspan, dd]
    C['XBpk'] = m
    C['XCpk'] = mc
    # M16 bf16 contraction consts (128, 64): [16d'+s, 8j+d] = 1 iff d'==d
    p_idx = np.arange(128)
    d_of_p = p_idx // 16
    for j in range(NCH):
        m = np.zeros((128, 64), f32)
        m[p_idx, 8 * j + d_of_p] = 1.0
        C[f'M16b_{j}'] = m
        # E8_j: (64, 128): [8j+d, 16d+s] = 1 (dt broadcast via PE)
        m = np.zeros((64, 128), f32)
        for d in range(8):
            m[8 * j + d, 16 * d:16 * d + 16] = 1.0
        C[f'E8_{j}'] = m
    # A column: p = 16d+s
    A = -np.exp(A_log)  # (8,16)
    C['Acol'] = A.reshape(128, 1).copy()
    # Dp diag
    m = np.zeros((64, 64), f32)
    np.fill_diagonal(m, np.tile(Dp, NCH))
    C['DpD'] = m
    # centered out_proj: Wc = (I - J/4) @ out_proj_w ; [8j+d, 4j+e] = Wc[e,d]
    Wc = ((np.eye(4, dtype=f32) - np.ones((4, 4), f32) / 4) @ out_proj_w).astype(f32)
    m = np.zeros((64, 32), f32)
    for j in range(NCH):
        m[8 * j:8 * j + 8, 4 * j:4 * j + 4] = Wc.T
    C['WcT'] = m
    # variance reduce: [4j+e, j] = 1/4
    m = np.zeros((32, 8), f32)
    for j in range(NCH):
        m[4 * j:4 * j + 4, j] = 0.25
    C['OnesV'] = m
    C['ndwb'] = -C['dwb']
    C['ncb'] = -C['cb']
    C['g32'] = np.tile(ln_g.reshape(1, 4), (NCH, 1)).reshape(32, 1).copy()
    C['eps8'] = np.full((8, 1), 1e-5, f32)
    C['zero64'] = np.zeros((64, 4), f32)
    C['ones'] = np.ones((128, 1), f32)
    C['cb128'] = np.concatenate([C['cb'], np.zeros((64, 1), f32)], axis=0)
    C['ncb128'] = -C['cb128']
    return C


# names of constants that are used as matmul lhsT (loaded as float32r)
_MM_CONSTS = (['dw0', 'dw1', 'dw2', 'Win', 'cvA', 'cvB', 'DD',
               'DpD', 'WcT', 'OnesV', 'zero64', 'XBpk', 'XCpk']
              + [f'E8_{j}' for j in range(NCH)])
_VEC_CONSTS = ['dwb', 'cb', 'dtb', 'Acol', 'g32', 'eps8', 'ndwb', 'ncb',
               'ones', 'cb128', 'ncb128']
_BF_CONSTS = [f'M16b_{j}' for j in range(NCH)]


def _pack_consts(consts):
    """Pack consts into two (128, N) blobs (f32r matmul weights / f32 vectors).
    Returns (blob_r, blob_v, layout) with layout[nm] = (which, rows, off, cols)."""
    layout = {}
    blobs = {'r': [], 'v': [], 'b': []}
    offs = {'r': 0, 'v': 0, 'b': 0}
    for which, names in (('r', _MM_CONSTS), ('v', _VEC_CONSTS), ('b', _BF_CONSTS)):
        for nm in names:
            a = consts[nm]
            rows, cols = a.shape
            pad = np.zeros((128, cols), np.float32)
            pad[:rows] = a
            layout[nm] = (which, rows, offs[which], cols)
            blobs[which].append(pad)
            offs[which] += cols
    import ml_dtypes
    blob_r = np.concatenate(blobs['r'], axis=1)
    blob_v = np.concatenate(blobs['v'], axis=1)
    blob_b = np.concatenate(blobs['b'], axis=1).astype(ml_dtypes.bfloat16)
    return blob_r, blob_v, blob_b, layout


def _build_program(layout_meta):
    import concourse.bass as bass
    import concourse.bacc as bacc
    import concourse.tile as tile
    import concourse.mybir as mybir
    import concourse.hw_specs as hw_specs
    import bass_rust

    F32 = mybir.dt.float32
    F32R = mybir.dt.float32r
    BF16 = mybir.dt.bfloat16
    AF = mybir.ActivationFunctionType
    OP = mybir.AluOpType

    # Force a single activation-function set (Exp/Ln/Square/Copy/Identity all
    # live in natural_log_exp_and_others) so bacc emits ONE table load instead
    # of thrashing between per-function sets. Positions are preserved so the
    # emitted act_func_set_id still indexes act_info.json correctly.
    real_tables = hw_specs.get_activation_tables("gen3")
    forced = {name: (fns if name == 'natural_log_exp_and_others' else set())
              for name, fns in real_tables.items()}
    bacc.get_activation_tables = lambda arch: forced

    nc = bacc.Bacc("TRN2", debug=False)

    seqs_d = nc.dram_tensor("seqs", [4, RL], F32R, kind="ExternalInput")
    blob_r_d = nc.dram_tensor("cblob_r", [128, layout_meta['nr']], F32R,
                              kind="ExternalInput")
    blob_v_d = nc.dram_tensor("cblob_v", [128, layout_meta['nv']], F32,
                              kind="ExternalInput")
    blob_b_d = nc.dram_tensor("cblob_b", [128, layout_meta['nb']], BF16,
                              kind="ExternalInput")
    oout_d = nc.dram_tensor("oout", [4, LH], F32, kind="ExternalOutput")

    def rep_ap(tile_ap, reps):
        """SBUF AP (p, n) -> read pattern replicating each partition row
        `reps` times consecutively (dest p = src_p*reps + r)."""
        src = tile_ap.copy()
        dims = [list(src.ap[i]) for i in range(len(src.ap))]
        src.ap = bass_rust.VecI64Pair([dims[0], [0, reps], dims[1]])
        return src

    with tile.TileContext(nc) as tc, ExitStack() as ctx:
        cpool = ctx.enter_context(tc.tile_pool(name="consts", bufs=1))
        blob_r = cpool.tile([128, layout_meta['nr']], F32R, tag="blob_r")
        nc.sync.dma_start(out=blob_r[:], in_=blob_r_d.ap())
        blob_v = cpool.tile([128, layout_meta['nv']], F32, tag="blob_v")
        nc.sync.dma_start(out=blob_v[:], in_=blob_v_d.ap())
        blob_b = cpool.tile([128, layout_meta['nb']], BF16, tag="blob_b")
        nc.sync.dma_start(out=blob_b[:], in_=blob_b_d.ap())
        CT = {}
        for nm, (which, rows, off, cols) in layout_meta['layout'].items():
            t = {'r': blob_r, 'v': blob_v, 'b': blob_b}[which]
            CT[nm] = t[0:rows, off:off + cols]

        sb = ctx.enter_context(tc.tile_pool(name="sb", bufs=2))
        sf = ctx.enter_context(tc.tile_pool(name="sf", bufs=3))
        sc = ctx.enter_context(tc.tile_pool(name="sc", bufs=2))
        hb = ctx.enter_context(tc.tile_pool(name="hb", bufs=10))
        ps_stage = ctx.enter_context(tc.tile_pool(name="ps_stage", bufs=4, space="PSUM"))
        ps_dtb = ctx.enter_context(tc.tile_pool(name="ps_dtb", bufs=1, space="PSUM"))
        ps_ys = ctx.enter_context(tc.tile_pool(name="ps_ys", bufs=1, space="PSUM"))

        ones = CT['ones']

        h_prev = [None] * NCH      # per-chunk previous h tile (bf16)
        xm_prev = None             # previous xm_s tile (3-col history carry)
        n_prev = 0

        for i, (g0, n) in enumerate(STEPS):
            burn = (i == 0)
            subs = [(0, n)] if n <= T else [(q * T, T) for q in range(n // T)]
            # ---- load input tile: (32, n+2) covering [g0-1, g0+n+1) ----
            s32 = sb.tile([32, n + 2], F32R, tag="s32")
            src = seqs_d.ap().copy()
            src.offset = (g0 - 1) + WBURN + 4
            src.ap = bass_rust.VecI64Pair([[LC, NCH], [RL, 4], [1, n + 2]])
            nc.gpsimd.dma_start(out=s32[:], in_=src)

            # step-level packed tiles (written in T-col halves)
            xm_s = sb.tile([128, n + 3], F32R, tag="xm_s")
            if i == 0:
                nc.vector.tensor_copy(xm_s[0:64, 0:3], CT['zero64'][0:64, 0:3])
                nc.vector.tensor_copy(xm_s[64:128, 0:1], CT['zero64'][0:64, 0:1])
            else:
                nc.vector.tensor_copy(xm_s[0:64, 0:3],
                                      xm_prev[0:64, n_prev:n_prev + 3])
                nc.vector.tensor_copy(xm_s[64:128, 0:1],
                                      xm_prev[0:64, n_prev + 2:n_prev + 3])
            X = sb.tile([128, n], F32R, tag="X")        # rows 0:64 xm2, 64:128 zs
            dt64 = sb.tile([64, n], F32R, tag="dt64")
            e1S = sb.tile([32, n], F32, tag="e1S")
            eCS = sb.tile([128, n], F32, tag="eCS")
            e4S = sb.tile([64, n], F32, tag="e4S")
            BmS = sb.tile([128, n], BF16, tag="BmS")
            if burn:
                CmS = None
            else:
                CmS = sb.tile([128, n], BF16, tag="CmS")

            # Phase A: dwconv + exp per sub-tile (v tiles stay alive)
            v_t = {}
            for (q0, m) in subs:
                v = ps_stage.tile([32, m], F32, tag="stage")
                for k in range(3):
                    nc.tensor.matmul(v[:], CT[f'dw{k}'], s32[:, q0 + k:q0 + k + m],
                                     start=(k == 0), stop=(k == 2))
                nc.scalar.activation(e1S[:, q0:q0 + m], v[:], AF.Exp, scale=-1.0,
                                     bias=CT['ndwb'][:, 0:1])
                v_t[q0] = v
            # Phase B: step-wide sigmoid tail for u
            l1S = sb.tile([32, n], F32, tag="l1S")
            nc.scalar.activation(l1S[:], e1S[:], AF.Ln, bias=ones[0:32, 0:1])
            m1S = sb.tile([32, n], F32, tag="m1S")
            nc.scalar.activation(m1S[:], l1S[:], AF.Exp, scale=-1.0)
            # Phase C: u, in_proj, conv, exp per sub-tile (P tiles stay alive)
            P_t = {}
            for (q0, m) in subs:
                u = sc.tile([32, m], F32R, tag="u")
                nc.vector.scalar_tensor_tensor(u[:], v_t[q0][:], CT['dwb'][:, 0:1],
                                               m1S[:, q0:q0 + m],
                                               op0=OP.add, op1=OP.mult)
                P = ps_stage.tile([128, m], F32, tag="stage")
                nc.tensor.matmul(P[:], CT['Win'], u[:], start=True, stop=True)
                nc.scalar.copy(xm_s[0:64, 3 + q0:3 + q0 + m], P[0:64, :])
                nc.scalar.copy(xm_s[64:128, 1 + q0:1 + q0 + m], P[0:64, :])
                nc.tensor.matmul(P[0:64, :], CT['cvA'], xm_s[:, q0:q0 + m],
                                 start=True, stop=False, skip_group_check=True)
                nc.tensor.matmul(P[0:64, :], CT['cvB'], xm_s[:, q0 + 1:q0 + 1 + m],
                                 start=False, stop=True, skip_group_check=True)
                nc.scalar.activation(eCS[:, q0:q0 + m], P[:], AF.Exp, scale=-1.0,
                                     bias=CT['ncb128'][:, 0:1])
                P_t[q0] = P
            # Phase D: step-wide sigmoid tail for (xm2; zs)
            lCS = sb.tile([128, n], F32, tag="lCS")
            nc.scalar.activation(lCS[:], eCS[:], AF.Ln, bias=ones[:, 0:1])
            mCS = sb.tile([128, n], F32, tag="mCS")
            nc.scalar.activation(mCS[:], lCS[:], AF.Exp, scale=-1.0)
            # Phase E: X, dt-pre, Bm/Cm per sub-tile
            for (q0, m) in subs:
                nc.vector.scalar_tensor_tensor(X[:, q0:q0 + m], P_t[q0][:],
                                               CT['cb128'][:, 0:1],
                                               mCS[:, q0:q0 + m],
                                               op0=OP.add, op1=OP.mult)
                dtp = ps_stage.tile([64, m], F32, tag="stage")
                nc.tensor.matmul(dtp[:], CT['DD'], X[0:64, q0:q0 + m],
                                 start=True, stop=True)
                nc.scalar.activation(e4S[:, q0:q0 + m], dtp[:], AF.Exp,
                                     bias=CT['dtb'][:, 0:1])
                Bm128 = ps_stage.tile([128, m], F32, tag="stage")
                nc.tensor.matmul(Bm128[:], CT['XBpk'], X[0:64, q0:q0 + m],
                                 start=True, stop=True)
                nc.scalar.copy(BmS[:, q0:q0 + m], Bm128[:])
                if not burn:
                    Cm128 = ps_stage.tile([128, m], F32, tag="stage")
                    nc.tensor.matmul(Cm128[:], CT['XCpk'], X[0:64, q0:q0 + m],
                                     start=True, stop=True)
                    nc.scalar.copy(CmS[:, q0:q0 + m], Cm128[:])
            # Phase F: step-wide softplus tail
            nc.scalar.activation(dt64[:], e4S[:], AF.Ln, bias=ones[0:64, 0:1])

            # w8 = dt * xm2 (bf16 out for the 2x dBx multiply)
            w8 = sb.tile([64, n], BF16, tag="w8")
            nc.vector.tensor_tensor(w8[:], dt64[:], X[0:64, :], op=OP.mult)

            # ---- ys accumulation target ----
            if not burn:
                ys = ps_ys.tile([64, n], F32, tag="ys")
                for (q0, m) in subs:
                    nc.tensor.matmul(ys[:, q0:q0 + m], CT['DpD'], X[0:64, q0:q0 + m],
                                     start=True, stop=False, skip_group_check=True)

            h_cur = [None] * NCH
            for j in range(NCH):
                r0 = 8 * j
                # dt broadcast on PE (fp32: dA precision matters)
                dtb_t = ps_dtb.tile([128, n], F32, tag="dtb")
                for (q0, m) in subs:
                    nc.tensor.matmul(dtb_t[:, q0:q0 + m], CT[f'E8_{j}'],
                                     dt64[:, q0:q0 + m], start=True, stop=True,
                                     skip_group_check=True)
                dA = sf.tile([128, n], BF16, tag="dA")
                nc.scalar.activation(dA[:], dtb_t[:], AF.Exp,
                                     scale=CT['Acol'][:, 0:1])
                w8b = sf.tile([128, n], BF16, tag="w8b")
                nc.gpsimd.dma_start(out=w8b[:], in_=rep_ap(w8[r0:r0 + 8, :], 16))
                Bmb = sb.tile([128, n], BF16, tag="Bmb")
                nc.sync.dma_start(out=Bmb[:],
                                  in_=rep_ap(BmS[16 * j:16 * j + 16, :], 8))
                dBx = sf.tile([128, n], BF16, tag="dBx")
                nc.vector.tensor_tensor(dBx[:], w8b[:], Bmb[:], op=OP.mult)
                h = hb.tile([128, n], BF16, tag="h")
                init = 0.0 if i == 0 else h_prev[j][:, n_prev - 1:n_prev]
                nc.vector.tensor_tensor_scan(h[:], dA[:], dBx[:], init,
                                             op0=OP.mult, op1=OP.add)
                h_cur[j] = h
                if not burn:
                    Cb = sb.tile([128, n], BF16, tag="Cb")
                    nc.gpsimd.dma_start(out=Cb[:],
                                        in_=rep_ap(CmS[16 * j:16 * j + 16, :], 8))
                    hc = sf.tile([128, n], BF16, tag="hc")
                    nc.vector.tensor_tensor(hc[:], h[:], Cb[:], op=OP.mult)
                    for (q0, m) in subs:
                        nc.tensor.matmul(ys[:, q0:q0 + m], CT[f'M16b_{j}'],
                                         hc[:, q0:q0 + m],
                                         start=False, stop=(j == NCH - 1),
                                         skip_group_check=True)
            h_prev = h_cur
            xm_prev = xm_s
            n_prev = n
            if burn:
                continue

            # ---- y = ys * silu(z) ----
            y = sb.tile([64, n], F32R, tag="y")
            nc.vector.tensor_tensor(y[:], X[64:128, :], ys[:], op=OP.mult)

            for (q0, m) in subs:
                outc = ps_stage.tile([32, m], F32, tag="stage")
                nc.tensor.matmul(outc[:], CT['WcT'], y[:, q0:q0 + m],
                                 start=True, stop=True)
                sq = sc.tile([32, m], F32R, tag="sq")
                nc.scalar.activation(sq[:], outc[:], AF.Square)
                var = ps_stage.tile([8, m], F32, tag="stage")
                nc.tensor.matmul(var[:], CT['OnesV'], sq[:], start=True, stop=True)
                lv = sc.tile([8, m], F32, tag="lv")
                nc.scalar.activation(lv[:], var[:], AF.Ln, bias=CT['eps8'][:, 0:1])
                r = sc.tile([8, m], F32, tag="r")
                nc.scalar.activation(r[:], lv[:], AF.Exp, scale=-0.5)
                r4s = sc.tile([32, m], F32, tag="r4s")
                nc.gpsimd.dma_start(out=r4s[:], in_=rep_ap(r[:, :], 4))
                o = sc.tile([32, m], F32, tag="o")
                nc.vector.scalar_tensor_tensor(o[:], outc[:], CT['g32'][:, 0:1],
                                               r4s[:], op0=OP.mult, op1=OP.mult)
                dst = oout_d.ap().copy()
                dst.offset = g0 + q0
                dst.ap = bass_rust.VecI64Pair([[LC, NCH], [LH, 4], [1, m]])
                nc.gpsimd.dma_start(out=dst, in_=o[:])

    nc.compile()
    return nc


def _host_gather(xb, perms, t0):
    """Gathered, padded input rows for one core: (4, RL) covering
    global times [t0-WBURN-4, t0+LH+1), zeros outside [0, L)."""
    xf = xb.reshape(L)
    lo = t0 - WBURN - 4
    out = np.zeros((4, RL), np.float32)
    ts = np.arange(lo, lo + RL)
    valid = (ts >= 0) & (ts < L)
    out[:, valid] = xf[perms[:, ts[valid]]]
    return out


def kernel(**inputs):
    from concourse.bass_utils import run_bass_kernel_spmd

    x = np.asarray(inputs['x'], np.float32)
    ln_b = np.asarray(inputs['ln_b'], np.float32)
    perms = _build_perms()
    consts = _build_consts(inputs)
    blob_r, blob_v, blob_b, layout = _pack_consts(consts)
    meta = {'nr': blob_r.shape[1], 'nv': blob_v.shape[1], 'nb': blob_b.shape[1],
            'layout': layout}

    nc = _build_program(meta)

    in_maps = []
    for c in range(N_CORES):
        b, half = c // 2, c % 2
        m = {'cblob_r': blob_r, 'cblob_v': blob_v, 'cblob_b': blob_b,
             'seqs': _host_gather(x[b], perms, half * LH)}
        in_maps.append(m)

    res = run_bass_kernel_spmd(nc, in_maps, core_ids=list(range(N_CORES)))

    # reassemble o (B, 4, L) in sequence domain
    o_full = np.empty((B, 4, L), np.float32)
    for c in range(N_CORES):
        b, half = c // 2, c % 2
        o_full[b, :, half * LH:(half + 1) * LH] = res.results[c]['oout']

    # final: result[b, l] = mean_i(o[b, i, perms[i, l]] + ln_b[i])
    out = np.zeros((B, L), np.float32)
    for i in range(4):
        out += o_full[:, i, :][:, perms[i]]
    out = out / 4 + ln_b.mean()
    return out.reshape(B, NF, HH, WW).astype(np.float32)


# revision 31
# speedup vs baseline: 1.0026x; 1.0026x over previous
"""Trainium2 Bass kernel for nn_FourScanBranch (4-direction Mamba scan over video).

Strategy:
- 8 cores; core c handles (batch = c // 2, L-half = c % 2). All sequence chunks
  are made independent via a short burn-in prefix (the scan decay is
  exp(-(s+1)*dt) with dt ~= 0.127, so influence dies to <1e-12 within ~128
  steps) -- no cross-core or cross-chunk serial dependency.
- Per core, the half-sequence (Lh = 49152) is processed as 8 interleaved
  chunks packed along SBUF partitions for the small-channel stages
  (4/8/16-channel convs + projections run 8 chunks wide), while the Mamba
  recurrence runs per chunk on the full 128 (d_inner x d_state) partition
  width via the DVE tensor_tensor_scan instruction.
- Host does the static center-out permutation gather/scatter (pure index
  shuffles baked from shapes) and the final 4-scan mean.
"""
import numpy as np
from contextlib import ExitStack

# ---------------- problem constants (hardcoded per contract) ----------------
B, NF, HH, WW = 4, 6, 128, 128
L = NF * HH * WW          # 98304
D_MODEL, D_INNER, D_STATE, D_CONV, DT_RANK = 4, 8, 16, 4, 1

N_CORES = 8
LH = L // 2               # 49152 per core
NCH = 8                   # chunks per core (packed along partitions)
LC = LH // NCH            # 6144
WBURN = 128               # burn-in columns per chunk
T = 512                   # matmul/PSUM sub-tile width
TSTEP = 1024              # step width (DVE/ACT/scan ops run this wide)
RL = WBURN + 4 + LH + 1   # per-channel row length of the gathered input buffer

# steps: (g0, n) in chunk-local time; step 0 is burn-in only
STEPS = [(-WBURN, WBURN)] + [(k * TSTEP, TSTEP) for k in range(LC // TSTEP)]


def _build_perms():
    yy, xx = np.meshgrid(np.arange(HH), np.arange(WW), indexing='ij')
    cy, cx = HH // 2, WW // 2
    d2 = (yy - cy) ** 2 + (xx - cx) ** 2
    sp = np.argsort(d2.reshape(-1), kind='stable')
    HW = HH * WW
    scanA = (np.arange(NF)[:, None] * HW + sp[None, :]).reshape(-1)
    fwd = np.arange(NF)
    bwd = fwd[::-1]
    tids = np.where((np.arange(HW) % 2 == 0)[:, None], fwd[None, :], bwd[None, :])
    scanB = (tids * HW + sp[:, None]).reshape(-1)
    return np.stack([scanA, scanB, scanA[::-1], scanB[::-1]], axis=0)  # (4, L)


def _build_consts(inputs):
    """Host-side constant matrices, keyed by dram tensor name."""
    f32 = np.float32
    dwconv_w = np.asarray(inputs['dwconv_w'], f32)   # (4,1,3)
    dwconv_b = np.asarray(inputs['dwconv_b'], f32)   # (4,)
    in_proj_w = np.asarray(inputs['in_proj_w'], f32)  # (16,4)
    conv1d_w = np.asarray(inputs['conv1d_w'], f32)   # (8,1,4)
    conv1d_b = np.asarray(inputs['conv1d_b'], f32)   # (8,)
    x_proj_w = np.asarray(inputs['x_proj_w'], f32)   # (33,8)
    dt_proj_w = np.asarray(inputs['dt_proj_w'], f32)  # (8,1)
    dt_proj_b = np.asarray(inputs['dt_proj_b'], f32)  # (8,)
    A_log = np.asarray(inputs['A_log'], f32)         # (8,16)
    Dp = np.asarray(inputs['Dp'], f32)               # (8,)
    out_proj_w = np.asarray(inputs['out_proj_w'], f32)  # (4,8)
    ln_g = np.asarray(inputs['ln_g'], f32)           # (4,)

    C = {}
    # dwconv diag matmuls (32 = 8 chunks x 4 ch, p = 4j+c)
    for k in range(3):
        m = np.zeros((32, 32), f32)
        np.fill_diagonal(m, np.tile(dwconv_w[:, 0, k], NCH))
        C[f'dw{k}'] = m
    C['dwb'] = np.tile(dwconv_b, NCH).reshape(32, 1)
    # in_proj: (32, 128): [4j+c, 8j+d]=W[d,c]; [4j+c, 64+8j+d]=W[8+d,c]
    m = np.zeros((32, 128), f32)
    for j in range(NCH):
        for c in range(4):
            for d in range(8):
                m[4 * j + c, 8 * j + d] = in_proj_w[d, c]
                m[4 * j + c, 64 + 8 * j + d] = in_proj_w[8 + d, c]
    C['Win'] = m
    # conv1d stacked matmuls: cvA = [diag(w0); diag(w2)], cvB = [diag(w1); diag(w3)]
    for nm, (ka, kb) in (('cvA', (0, 2)), ('cvB', (1, 3))):
        m = np.zeros((128, 64), f32)
        np.fill_diagonal(m[0:64], np.tile(conv1d_w[:, 0, ka], NCH))
        np.fill_diagonal(m[64:128], np.tile(conv1d_w[:, 0, kb], NCH))
        C[nm] = m
    C['cb'] = np.tile(conv1d_b, NCH).reshape(64, 1)
    # dt_pre fused: [8j+dd, 8j+d] = x_proj_w[0,dd] * dt_proj_w[d,0]
    m = np.zeros((64, 64), f32)
    blk = np.outer(x_proj_w[0, :], dt_proj_w[:, 0]).astype(f32)  # (dd, d)
    for j in range(NCH):
        m[8 * j:8 * j + 8, 8 * j:8 * j + 8] = blk
    C['DD'] = m
    C['dtb'] = np.tile(dt_proj_b, NCH).reshape(64, 1)
    # packed x_proj fused broadcasts: Bm128[16j+s, t] = sum_dd xp[1+s,dd]*xm2[8j+dd,t]
    m = np.zeros((64, 128), f32)
    mc = np.zeros((64, 128), f32)
    for j in range(NCH):
        for dd in range(8):
            for sspan in range(16):
                m[8 * j + dd, 16 * j + sspan] = x_proj_w[1 + sspan, dd]
                mc[8 * j + dd, 16 * j + sspan] = x_proj_w[17 + sspan, dd]
    C['XBpk'] = m
    C['XCpk'] = mc
    # M16 bf16 contraction consts (128, 64): [16d'+s, 8j+d] = 1 iff d'==d
    p_idx = np.arange(128)
    d_of_p = p_idx // 16
    for j in range(NCH):
        m = np.zeros((128, 64), f32)
        m[p_idx, 8 * j + d_of_p] = 1.0
        C[f'M16b_{j}'] = m
        # E8_j: (64, 128): [8j+d, 16d+s] = 1 (dt broadcast via PE)
        m = np.zeros((64, 128), f32)
        for d in range(8):
            m[8 * j + d, 16 * d:16 * d + 16] = 1.0
        C[f'E8_{j}'] = m
    # A column: p = 16d+s
    A = -np.exp(A_log)  # (8,16)
    C['Acol'] = A.reshape(128, 1).copy()
    # Dp diag
    m = np.zeros((64, 64), f32)
    np.fill_diagonal(m, np.tile(Dp, NCH))
    C['DpD'] = m
    # centered out_proj: Wc = (I - J/4) @ out_proj_w ; [8j+d, 4j+e] = Wc[e,d]
    Wc = ((np.eye(4, dtype=f32) - np.ones((4, 4), f32) / 4) @ out_proj_w).astype(f32)
    m = np.zeros((64, 32), f32)
    for j in range(NCH):
        m[8 * j:8 * j + 8, 4 * j:4 * j + 4] = Wc.T
    C['WcT'] = m
    # variance reduce: [4j+e, j] = 1/4
    m = np.zeros((32, 8), f32)
    for j in range(NCH):
        m[4 * j:4 * j + 4, j] = 0.25
    C['OnesV'] = m
    C['ndwb'] = -C['dwb']
    C['ncb'] = -C['cb']
    C['g32'] = np.tile(ln_g.reshape(1, 4), (NCH, 1)).reshape(32, 1).copy()
    C['eps8'] = np.full((8, 1), 1e-5, f32)
    C['zero64'] = np.zeros((64, 4), f32)
    C['ones'] = np.ones((128, 1), f32)
    C['cb128'] = np.concatenate([C['cb'], np.zeros((64, 1), f32)], axis=0)
    C['ncb128'] = -C['cb128']
    return C


# names of constants that are used as matmul lhsT (loaded as float32r)
_MM_CONSTS = (['dw0', 'dw1', 'dw2', 'Win', 'cvA', 'cvB', 'DD',
               'DpD', 'WcT', 'OnesV', 'zero64', 'XBpk', 'XCpk']
              + [f'E8_{j}' for j in range(NCH)])
_VEC_CONSTS = ['dwb', 'cb', 'dtb', 'Acol', 'g32', 'eps8', 'ndwb', 'ncb',
               'ones', 'cb128', 'ncb128']
_BF_CONSTS = [f'M16b_{j}' for j in range(NCH)]


def _pack_consts(consts):
    """Pack consts into two (128, N) blobs (f32r matmul weights / f32 vectors).
    Returns (blob_r, blob_v, layout) with layout[nm] = (which, rows, off, cols)."""
    layout = {}
    blobs = {'r': [], 'v': [], 'b': []}
    offs = {'r': 0, 'v': 0, 'b': 0}
    for which, names in (('r', _MM_CONSTS), ('v', _VEC_CONSTS), ('b', _BF_CONSTS)):
        for nm in names:
            a = consts[nm]
            rows, cols = a.shape
            pad = np.zeros((128, cols), np.float32)
            pad[:rows] = a
            layout[nm] = (which, rows, offs[which], cols)
            blobs[which].append(pad)
            offs[which] += cols
    import ml_dtypes
    blob_r = np.concatenate(blobs['r'], axis=1)
    blob_v = np.concatenate(blobs['v'], axis=1)
    blob_b = np.concatenate(blobs['b'], axis=1).astype(ml_dtypes.bfloat16)
    return blob_r, blob_v, blob_b, layout


def _build_program(layout_meta):
    import concourse.bass as bass
    import concourse.bacc as bacc
    import concourse.tile as tile
    import concourse.mybir as mybir
    import concourse.hw_specs as hw_specs
    import bass_rust

    F32 = mybir.dt.float32
    F32R = mybir.dt.float32r
    BF16 = mybir.dt.bfloat16
    AF = mybir.ActivationFunctionType
    OP = mybir.AluOpType

    # Force a single activation-function set (Exp/Ln/Square/Copy/Identity all
    # live in natural_log_exp_and_others) so bacc emits ONE table load instead
    # of thrashing between per-function sets. Positions are preserved so the
    # emitted act_func_set_id still indexes act_info.json correctly.
    real_tables = hw_specs.get_activation_tables("gen3")
    forced = {name: (fns if name == 'natural_log_exp_and_others' else set())
              for name, fns in real_tables.items()}
    bacc.get_activation_tables = lambda arch: forced

    nc = bacc.Bacc("TRN2", debug=False)

    seqs_d = nc.dram_tensor("seqs_v9", [4, RL], F32R, kind="ExternalInput")
    blob_r_d = nc.dram_tensor("cblob_r", [128, layout_meta['nr']], F32R,
                              kind="ExternalInput")
    blob_v_d = nc.dram_tensor("cblob_v", [128, layout_meta['nv']], F32,
                              kind="ExternalInput")
    blob_b_d = nc.dram_tensor("cblob_b", [128, layout_meta['nb']], BF16,
                              kind="ExternalInput")
    oout_d = nc.dram_tensor("oout", [4, LH], F32, kind="ExternalOutput")

    def rep_ap(tile_ap, reps):
        """SBUF AP (p, n) -> read pattern replicating each partition row
        `reps` times consecutively (dest p = src_p*reps + r)."""
        src = tile_ap.copy()
        dims = [list(src.ap[i]) for i in range(len(src.ap))]
        src.ap = bass_rust.VecI64Pair([dims[0], [0, reps], dims[1]])
        return src

    with tile.TileContext(nc) as tc, ExitStack() as ctx:
        cpool = ctx.enter_context(tc.tile_pool(name="consts", bufs=1))
        blob_r = cpool.tile([128, layout_meta['nr']], F32R, tag="blob_r")
        nc.sync.dma_start(out=blob_r[:], in_=blob_r_d.ap())
        blob_v = cpool.tile([128, layout_meta['nv']], F32, tag="blob_v")
        nc.sync.dma_start(out=blob_v[:], in_=blob_v_d.ap())
        blob_b = cpool.tile([128, layout_meta['nb']], BF16, tag="blob_b")
        nc.sync.dma_start(out=blob_b[:], in_=blob_b_d.ap())
        CT = {}
        for nm, (which, rows, off, cols) in layout_meta['layout'].items():
            t = {'r': blob_r, 'v': blob_v, 'b': blob_b}[which]
            CT[nm] = t[0:rows, off:off + cols]

        sb = ctx.enter_context(tc.tile_pool(name="sb", bufs=2))
        sf = ctx.enter_context(tc.tile_pool(name="sf", bufs=3))
        sc = ctx.enter_context(tc.tile_pool(name="sc", bufs=2))
        hb = ctx.enter_context(tc.tile_pool(name="hb", bufs=10))
        ps_stage = ctx.enter_context(tc.tile_pool(name="ps_stage", bufs=4, space="PSUM"))
        ps_dtb = ctx.enter_context(tc.tile_pool(name="ps_dtb", bufs=1, space="PSUM"))
        ps_ys = ctx.enter_context(tc.tile_pool(name="ps_ys", bufs=1, space="PSUM"))

        ones = CT['ones']

        h_prev = [None] * NCH      # per-chunk previous h tile (bf16)
        xm_prev = None             # previous xm_s tile (3-col history carry)
        n_prev = 0

        for i, (g0, n) in enumerate(STEPS):
            burn = (i == 0)
            subs = [(0, n)] if n <= T else [(q * T, T) for q in range(n // T)]
            # ---- load input tile: (32, n+2) covering [g0-1, g0+n+1) ----
            s32 = sb.tile([32, n + 2], F32R, tag="s32")
            src = seqs_d.ap().copy()
            src.offset = (g0 - 1) + WBURN + 4
            src.ap = bass_rust.VecI64Pair([[LC, NCH], [RL, 4], [1, n + 2]])
            nc.gpsimd.dma_start(out=s32[:], in_=src)

            # step-level packed tiles (written in T-col halves)
            xm_s = sb.tile([128, n + 3], F32R, tag="xm_s")
            if i == 0:
                nc.vector.tensor_copy(xm_s[0:64, 0:3], CT['zero64'][0:64, 0:3])
                nc.vector.tensor_copy(xm_s[64:128, 0:1], CT['zero64'][0:64, 0:1])
            else:
                nc.vector.tensor_copy(xm_s[0:64, 0:3],
                                      xm_prev[0:64, n_prev:n_prev + 3])
                nc.vector.tensor_copy(xm_s[64:128, 0:1],
                                      xm_prev[0:64, n_prev + 2:n_prev + 3])
            X = sb.tile([128, n], F32R, tag="X")        # rows 0:64 xm2, 64:128 zs
            dt64 = sb.tile([64, n], F32R, tag="dt64")
            e1S = sb.tile([32, n], F32, tag="e1S")
            eCS = sb.tile([128, n], F32, tag="eCS")
            e4S = sb.tile([64, n], F32, tag="e4S")
            BmS = sb.tile([128, n], BF16, tag="BmS")
            if burn:
                CmS = None
            else:
                CmS = sb.tile([128, n], BF16, tag="CmS")

            # Phase A: dwconv + exp per sub-tile (v tiles stay alive)
            v_t = {}
            for (q0, m) in subs:
                v = ps_stage.tile([32, m], F32, tag="stage")
                for k in range(3):
                    nc.tensor.matmul(v[:], CT[f'dw{k}'], s32[:, q0 + k:q0 + k + m],
                                     start=(k == 0), stop=(k == 2))
                nc.scalar.activation(e1S[:, q0:q0 + m], v[:], AF.Exp, scale=-1.0,
                                     bias=CT['ndwb'][:, 0:1])
                v_t[q0] = v
            # Phase B: step-wide sigmoid tail for u
            l1S = sb.tile([32, n], F32, tag="l1S")
            nc.scalar.activation(l1S[:], e1S[:], AF.Ln, bias=ones[0:32, 0:1])
            m1S = sb.tile([32, n], F32, tag="m1S")
            nc.scalar.activation(m1S[:], l1S[:], AF.Exp, scale=-1.0)
            # Phase C: u, in_proj, conv, exp per sub-tile (P tiles stay alive)
            P_t = {}
            for (q0, m) in subs:
                u = sc.tile([32, m], F32R, tag="u")
                nc.vector.scalar_tensor_tensor(u[:], v_t[q0][:], CT['dwb'][:, 0:1],
                                               m1S[:, q0:q0 + m],
                                               op0=OP.add, op1=OP.mult)
                P = ps_stage.tile([128, m], F32, tag="stage")
                nc.tensor.matmul(P[:], CT['Win'], u[:], start=True, stop=True)
                nc.scalar.copy(xm_s[0:64, 3 + q0:3 + q0 + m], P[0:64, :])
                nc.scalar.copy(xm_s[64:128, 1 + q0:1 + q0 + m], P[0:64, :])
                nc.tensor.matmul(P[0:64, :], CT['cvA'], xm_s[:, q0:q0 + m],
                                 start=True, stop=False, skip_group_check=True)
                nc.tensor.matmul(P[0:64, :], CT['cvB'], xm_s[:, q0 + 1:q0 + 1 + m],
                                 start=False, stop=True, skip_group_check=True)
                nc.scalar.activation(eCS[:, q0:q0 + m], P[:], AF.Exp, scale=-1.0,
                                     bias=CT['ncb128'][:, 0:1])
                P_t[q0] = P
            # Phase D: step-wide sigmoid tail for (xm2; zs)
            lCS = sb.tile([128, n], F32, tag="lCS")
            nc.scalar.activation(lCS[:], eCS[:], AF.Ln, bias=ones[:, 0:1])
            mCS = sb.tile([128, n], F32, tag="mCS")
            nc.scalar.activation(mCS[:], lCS[:], AF.Exp, scale=-1.0)
            # Phase E: X, dt-pre, Bm/Cm per sub-tile
            for (q0, m) in subs:
                nc.vector.scalar_tensor_tensor(X[:, q0:q0 + m], P_t[q0][:],
                                               CT['cb128'][:, 0:1],
                                               mCS[:, q0:q0 + m],
                                               op0=OP.add, op1=OP.mult)
                dtp = ps_stage.tile([64, m], F32, tag="stage")
                nc.tensor.matmul(dtp[:], CT['DD'], X[0:64, q0:q0 + m],
                                 start=True, stop=True)
                nc.scalar.activation(e4S[:, q0:q0 + m], dtp[:], AF.Exp,
                                     bias=CT['dtb'][:, 0:1])
                Bm128 = ps_stage.tile([128, m], F32, tag="stage")
                nc.tensor.matmul(Bm128[:], CT['XBpk'], X[0:64, q0:q0 + m],
                                 start=True, stop=True)
                nc.scalar.copy(BmS[:, q0:q0 + m], Bm128[:])
                if not burn:
                    Cm128 = ps_stage.tile([128, m], F32, tag="stage")
                    nc.tensor.matmul(Cm128[:], CT['XCpk'], X[0:64, q0:q0 + m],
                                     start=True, stop=True)
                    nc.scalar.copy(CmS[:, q0:q0 + m], Cm128[:])
            # Phase F: step-wide softplus tail
            nc.scalar.activation(dt64[:], e4S[:], AF.Ln, bias=ones[0:64, 0:1])

            # w8 = dt * xm2 (bf16 out for the 2x dBx multiply)
            w8 = sb.tile([64, n], BF16, tag="w8")
            nc.vector.tensor_tensor(w8[:], dt64[:], X[0:64, :], op=OP.mult)

            # ---- ys accumulation target ----
            if not burn:
                ys = ps_ys.tile([64, n], F32, tag="ys")
                for (q0, m) in subs:
                    nc.tensor.matmul(ys[:, q0:q0 + m], CT['DpD'], X[0:64, q0:q0 + m],
                                     start=True, stop=False, skip_group_check=True)

            h_cur = [None] * NCH
            for j in range(NCH):
                r0 = 8 * j
                # dt broadcast on PE (fp32: dA precision matters)
                dtb_t = ps_dtb.tile([128, n], F32, tag="dtb")
                for (q0, m) in subs:
                    nc.tensor.matmul(dtb_t[:, q0:q0 + m], CT[f'E8_{j}'],
                                     dt64[:, q0:q0 + m], start=True, stop=True,
                                     skip_group_check=True)
                dA = sf.tile([128, n], BF16, tag="dA")
                nc.scalar.activation(dA[:], dtb_t[:], AF.Exp,
                                     scale=CT['Acol'][:, 0:1])
                w8b = sf.tile([128, n], BF16, tag="w8b")
                nc.gpsimd.dma_start(out=w8b[:], in_=rep_ap(w8[r0:r0 + 8, :], 16))
                Bmb = sb.tile([128, n], BF16, tag="Bmb")
                nc.sync.dma_start(out=Bmb[:],
                                  in_=rep_ap(BmS[16 * j:16 * j + 16, :], 8))
                dBx = sf.tile([128, n], BF16, tag="dBx")
                nc.vector.tensor_tensor(dBx[:], w8b[:], Bmb[:], op=OP.mult)
                h = hb.tile([128, n], BF16, tag="h")
                init = 0.0 if i == 0 else h_prev[j][:, n_prev - 1:n_prev]
                nc.vector.tensor_tensor_scan(h[:], dA[:], dBx[:], init,
                                             op0=OP.mult, op1=OP.add)
                h_cur[j] = h
                if not burn:
                    Cb = sb.tile([128, n], BF16, tag="Cb")
                    nc.gpsimd.dma_start(out=Cb[:],
                                        in_=rep_ap(CmS[16 * j:16 * j + 16, :], 8))
                    hc = sf.tile([128, n], BF16, tag="hc")
                    nc.vector.tensor_tensor(hc[:], h[:], Cb[:], op=OP.mult)
                    for (q0, m) in subs:
                        nc.tensor.matmul(ys[:, q0:q0 + m], CT[f'M16b_{j}'],
                                         hc[:, q0:q0 + m],
                                         start=False, stop=(j == NCH - 1),
                                         skip_group_check=True)
            h_prev = h_cur
            xm_prev = xm_s
            n_prev = n
            if burn:
                continue

            # ---- y = ys * silu(z) ----
            y = sb.tile([64, n], F32R, tag="y")
            nc.vector.tensor_tensor(y[:], X[64:128, :], ys[:], op=OP.mult)

            for (q0, m) in subs:
                outc = ps_stage.tile([32, m], F32, tag="stage")
                nc.tensor.matmul(outc[:], CT['WcT'], y[:, q0:q0 + m],
                                 start=True, stop=True)
                sq = sc.tile([32, m], F32R, tag="sq")
                nc.scalar.activation(sq[:], outc[:], AF.Square)
                var = ps_stage.tile([8, m], F32, tag="stage")
                nc.tensor.matmul(var[:], CT['OnesV'], sq[:], start=True, stop=True)
                lv = sc.tile([8, m], F32, tag="lv")
                nc.scalar.activation(lv[:], var[:], AF.Ln, bias=CT['eps8'][:, 0:1])
                r = sc.tile([8, m], F32, tag="r")
                nc.scalar.activation(r[:], lv[:], AF.Exp, scale=-0.5)
                r4s = sc.tile([32, m], F32, tag="r4s")
                nc.gpsimd.dma_start(out=r4s[:], in_=rep_ap(r[:, :], 4))
                o = sc.tile([32, m], F32, tag="o")
                nc.vector.scalar_tensor_tensor(o[:], outc[:], CT['g32'][:, 0:1],
                                               r4s[:], op0=OP.mult, op1=OP.mult)
                dst = oout_d.ap().copy()
                dst.offset = g0 + q0
                dst.ap = bass_rust.VecI64Pair([[LC, NCH], [LH, 4], [1, m]])
                nc.gpsimd.dma_start(out=dst, in_=o[:])

    nc.compile()
    return nc


def _host_gather(xb, perms, t0):
    """Gathered, padded input rows for one core: (4, RL) covering
    global times [t0-WBURN-4, t0+LH+1), zeros outside [0, L)."""
    xf = xb.reshape(L)
    lo = t0 - WBURN - 4
    out = np.zeros((4, RL), np.float32)
    ts = np.arange(lo, lo + RL)
    valid = (ts >= 0) & (ts < L)
    out[:, valid] = xf[perms[:, ts[valid]]]
    return out


def kernel(**inputs):
    from concourse.bass_utils import run_bass_kernel_spmd

    x = np.asarray(inputs['x'], np.float32)
    ln_b = np.asarray(inputs['ln_b'], np.float32)
    perms = _build_perms()
    consts = _build_consts(inputs)
    blob_r, blob_v, blob_b, layout = _pack_consts(consts)
    meta = {'nr': blob_r.shape[1], 'nv': blob_v.shape[1], 'nb': blob_b.shape[1],
            'layout': layout}

    nc = _build_program(meta)

    in_maps = []
    for c in range(N_CORES):
        b, half = c // 2, c % 2
        m = {'cblob_r': blob_r, 'cblob_v': blob_v, 'cblob_b': blob_b,
             'seqs_v9': _host_gather(x[b], perms, half * LH)}
        in_maps.append(m)

    res = run_bass_kernel_spmd(nc, in_maps, core_ids=list(range(N_CORES)))

    # reassemble o (B, 4, L) in sequence domain
    o_full = np.empty((B, 4, L), np.float32)
    for c in range(N_CORES):
        b, half = c // 2, c % 2
        o_full[b, :, half * LH:(half + 1) * LH] = res.results[c]['oout']

    # final: result[b, l] = mean_i(o[b, i, perms[i, l]] + ln_b[i])
    out = np.zeros((B, L), np.float32)
    for i in range(4):
        out += o_full[:, i, :][:, perms[i]]
    out = out / 4 + ln_b.mean()
    return out.reshape(B, NF, HH, WW).astype(np.float32)
